# revision 1
# baseline (speedup 1.0000x reference)
"""Multi-head attention (B=8, N=1024, C=1024, H=16) on 8 Trainium2 NeuronCores.

Sharding: pure data-parallel — one batch element per core, weights replicated,
no collectives.

Per-core algorithm (all matmuls fp32r on the PE at full rate):
  phase 1a: qk projection.  qkT[d, n] = wqkT[c, d].T @ xT[c, n], d in [0, 2048).
            Bounced through DRAM (SBUF pressure) for per-head streaming later.
  phase 1b: v projection into *natural* [m, d_v] layout, stored interleaved as
            [m, 16*(64+1)] with a ones-column appended per head (the ones
            column makes the PV matmul emit softmax denominators for free).
  phase 2:  per head pair (row-packed K=64 matmuls on disjoint partition
            halves):
              S^T[m, n] = kT.T @ qT          (keys on partitions)
              expS = exp(SCALE * S^T)        (ACT, straight out of PSUM)
              U_aug[65, n] = v_aug.T @ expS  (rows 0..63 = unnormalized PV,
                                              row 64 = softmax denominator)
            denominators: gather -> reciprocal_approx_fast -> DRAM ->
            partition-broadcast DMA -> fused normalize (tensor_mul) into the
            attention-output-transposed buffer attn_outT[c, n].
  phase 3:  out[n, d] = attn_outT[c, n].T @ wpT[c, d] + b  (bias via
            broadcast tile + tensor_add), DMA to DRAM.
"""

import sys

if "/opt/trn_rl_repo" not in sys.path:
    sys.path.insert(0, "/opt/trn_rl_repo")

from contextlib import ExitStack

import numpy as np

import concourse.bass as bass
import concourse.mybir as mybir
from concourse import bacc
import concourse.tile as tile
from concourse import bass_utils

B, N, C, H = 8, 1024, 1024, 16
HD = C // H          # 64
SCALE = HD ** -0.5   # 0.125
P = 128              # SBUF partitions
NT = 512             # moving-dim tile (fp32 PSUM bank limit)
NCH = C // P         # 8 contraction chunks over channels
NMT = N // P         # 8 token tiles of 128
NNT = N // NT        # 2 token tiles of 512
F32 = mybir.dt.float32
F32R = mybir.dt.float32r
EXP = mybir.ActivationFunctionType.Exp


def build_module():
    import os
    _SKIP = set(filter(None, os.environ.get("K_SKIP", "").split(",")))
    nc = bacc.Bacc("TRN2", target_bir_lowering=False, debug=False, num_devices=B)

    xT = nc.dram_tensor("xT", [C, N], F32R, kind="ExternalInput").ap()
    wqkT = nc.dram_tensor("wqkT", [C, 2 * C], F32R, kind="ExternalInput").ap()
    wvT = nc.dram_tensor("wvT", [C, C], F32R, kind="ExternalInput").ap()
    wpT = nc.dram_tensor("wpT", [C, C], F32R, kind="ExternalInput").ap()
    bias = nc.dram_tensor("bias_bc", [P, C], F32, kind="ExternalInput").ap()
    ones_col = nc.dram_tensor("ones_col", [P, H], F32R, kind="ExternalInput").ap()
    out = nc.dram_tensor("out", [N, C], F32, kind="ExternalOutput").ap()

    with tile.TileContext(nc) as tc, ExitStack() as ctx:
        dram = ctx.enter_context(tc.tile_pool(name="dram", bufs=1, space="DRAM"))
        qkT_d = dram.tile([2 * C, N], F32R, tag="qkT_d", name="qkT_d")
        rden_d = dram.tile([H * NNT, NT], F32, tag="rden_d", name="rden_d")

        # 8 slots of [128, 1024]: first holds xT, later recycled for expS^T.
        xt_pool = ctx.enter_context(tc.tile_pool(name="xt", bufs=8))
        wqk_pool = ctx.enter_context(tc.tile_pool(name="wqk", bufs=10))
        wst_pool = ctx.enter_context(tc.tile_pool(name="wst", bufs=10))
        stage_pool = ctx.enter_context(tc.tile_pool(name="stage", bufs=4))
        vsb_pool = ctx.enter_context(tc.tile_pool(name="vsb", bufs=1))
        aot_pool = ctx.enter_context(tc.tile_pool(name="aot", bufs=1))
        qkp_pool = ctx.enter_context(tc.tile_pool(name="qkp", bufs=4))
        usb_pool = ctx.enter_context(tc.tile_pool(name="usb", bufs=6))
        den_pool = ctx.enter_context(tc.tile_pool(name="den", bufs=3))
        rbc_pool = ctx.enter_context(tc.tile_pool(name="rbc", bufs=6))
        one_pool = ctx.enter_context(tc.tile_pool(name="one", bufs=1))
        psum = ctx.enter_context(tc.tile_pool(name="psum", bufs=5, space="PSUM"))
        psum_u = ctx.enter_context(tc.tile_pool(name="psum_u", bufs=2, space="PSUM"))

        # ---------- input loads ----------
        xts = []
        for t in range(NCH):
            xt_t = xt_pool.tile([P, N], F32R, tag="xt", name=f"xt{t}")
            nc.sync.dma_start(xt_t, xT[t * P : (t + 1) * P, :])
            xts.append(xt_t)
        bias_sb = one_pool.tile([P, C], F32, tag="bias", name="bias_sb")
        nc.sync.dma_start(bias_sb, bias)

        if "pv" not in _SKIP:
            # ---------- phase 1b: v projection (natural layout + ones cols) ----------
            vsb = []
            for mt in range(NMT):
                v_t = vsb_pool.tile([P, H * (HD + 1)], F32R, tag=f"v{mt}", name=f"v{mt}")
                nc.sync.dma_start(
                    v_t.rearrange("p (h w) -> p h w", w=HD + 1)[:, :, HD : HD + 1], ones_col
                )
                vsb.append(v_t)
            for dvt in range(NNT):  # two 512-wide halves of d_v (heads 8*dvt..8*dvt+7)
                wv_tiles = []
                for ck in range(NCH):
                    wv_t = wst_pool.tile([P, NT], F32R, tag="wst", name=f"wv{dvt}_{ck}")
                    nc.sync.dma_start(
                        wv_t, wvT[ck * P : (ck + 1) * P, dvt * NT : (dvt + 1) * NT]
                    )
                    wv_tiles.append(wv_t)
                for mt in range(NMT):
                    ps_v = psum.tile([P, NT], F32, tag="ps", name=f"psv{mt}_{dvt}")
                    for ck in range(NCH):
                        nc.tensor.matmul(
                            ps_v,
                            lhsT=(xts[ck][:, mt * P : (mt + 1) * P]),
                            rhs=(wv_tiles[ck]),
                            start=(ck == 0),
                            stop=(ck == NCH - 1),
                        )
                    dst = vsb[mt].rearrange("p (h w) -> p h w", w=HD + 1)[
                        :, dvt * 8 : (dvt + 1) * 8, 0:HD
                    ]
                    nc.vector.tensor_copy(dst, ps_v.rearrange("p (h w) -> p h w", w=HD))

        if "pqk" not in _SKIP:
            # ---------- phase 1a: qk projection -> DRAM bounce ----------
            for dg in range(8):  # 256-wide d-groups over the 2048 qk channels
                wq_tiles = []
                for ck in range(NCH):
                    wq_t = wqk_pool.tile([P, 256], F32R, tag="wqk", name=f"wq{dg}_{ck}")
                    nc.sync.dma_start(
                        wq_t, wqkT[ck * P : (ck + 1) * P, dg * 256 : (dg + 1) * 256]
                    )
                    wq_tiles.append(wq_t)
                for ds_ in range(2):
                    dt = dg * 2 + ds_
                    for nt in range(NNT):
                        ps_qk = psum.tile([P, NT], F32, tag="ps", name=f"psqk{dt}_{nt}")
                        for ck in range(NCH):
                            nc.tensor.matmul(
                                ps_qk,
                                lhsT=(wq_tiles[ck][:, ds_ * P : (ds_ + 1) * P]),
                                rhs=(xts[ck][:, nt * NT : (nt + 1) * NT]),
                                start=(ck == 0),
                                stop=(ck == NCH - 1),
                            )
                        st = stage_pool.tile([P, NT], F32R, tag="stage", name=f"st{dt}_{nt}")
                        nc.vector.tensor_copy(st, ps_qk)
                        nc.sync.dma_start(
                            qkT_d[dt * P : (dt + 1) * P, nt * NT : (nt + 1) * NT], st
                        )

        # ---------- attention-output accumulator (attn_outT[c, n]) ----------
        aot = []
        for t in range(NCH):
            a_t = aot_pool.tile([P, N], F32R, tag=f"aot{t}", name=f"aot{t}")
            aot.append(a_t)

        if "pattn" in _SKIP and "pproj" not in _SKIP:
            for t in range(NCH):
                nc.sync.dma_start(aot[t], xT[t * P : (t + 1) * P, :])
        if "pattn" not in _SKIP:
            # ---------- phase 2: attention, software-pipelined ----------
            # Per unit u=(head, nt): S^T matmuls + exp.  PV of unit u-1 is
            # emitted between S(u) and S(u+1) so the PE fills the exp tail.
            units = []
            for pair in range(H // 2):
                hA = 2 * pair
                qp = qkp_pool.tile([P, N], F32R, tag="qp", name=f"qp{pair}")
                nc.sync.dma_start(qp, qkT_d[hA * HD : hA * HD + P, :])
                kp = qkp_pool.tile([P, N], F32R, tag="kp", name=f"kp{pair}")
                nc.sync.dma_start(kp, qkT_d[C + hA * HD : C + hA * HD + P, :])
                for j in range(2):
                    for nt in range(NNT):
                        units.append((pair, hA + j, j, nt, qp, kp))

            def emit_s_exp(u):
                (pair, h, j, nt, qp, kp) = u
                pl = slice(j * HD, (j + 1) * HD)
                exps = [
                    xt_pool.tile([P, N], F32R, tag="xt", name=f"e{h}_{nt}_{q}")
                    for q in range(4)
                ]
                for mc in range(NMT):
                    ps_s = psum.tile([P, NT], F32, tag="ps", name=f"pss{h}_{nt}_{mc}")
                    nc.tensor.matmul(
                        ps_s,
                        lhsT=kp[pl, mc * P : (mc + 1) * P],
                        rhs=qp[pl, nt * NT : (nt + 1) * NT],
                        start=True,
                        stop=True,
                    )
                    nc.scalar.activation(
                        exps[mc // 2][:, (mc % 2) * NT : (mc % 2 + 1) * NT],
                        ps_s,
                        EXP,
                        scale=SCALE,
                    )
                return exps

            pair_done = {}

            def emit_pv(u, exps):
                (pair, h, j, nt, qp, kp) = u
                ps_u = psum_u.tile([HD + 1, NT], F32, tag="pu", name=f"psu{h}_{nt}")
                for mc in range(NMT):
                    nc.tensor.matmul(
                        ps_u,
                        lhsT=vsb[mc][:, h * (HD + 1) : (h + 1) * (HD + 1)],
                        rhs=exps[mc // 2][:, (mc % 2) * NT : (mc % 2 + 1) * NT],
                        start=(mc == 0),
                        stop=(mc == NMT - 1),
                    )
                u_sb = usb_pool.tile([HD + 1, NT], F32, tag="usb", name=f"u{h}_{nt}")
                nc.vector.tensor_copy(u_sb, ps_u)
                pair_done.setdefault(pair, []).append((h, nt, u_sb))
                if len(pair_done[pair]) == 4:
                    emit_denoms(pair, pair_done.pop(pair))

            def emit_denoms(pair, punits):
                den_g = den_pool.tile([4, NT], F32, tag="den", name=f"den{pair}")
                for i, (h, nt, u_sb) in enumerate(punits):
                    nc.sync.dma_start(den_g[i : i + 1, :], u_sb[HD : HD + 1, :])
                rden = den_pool.tile([4, NT], F32, tag="rden", name=f"rden{pair}")
                nc.vector.reciprocal_approx_fast(out=rden, in_=den_g)
                nc.sync.dma_start(rden_d[pair * 4 : pair * 4 + 4, :], rden)
                for i, (h, nt, u_sb) in enumerate(punits):
                    rbc = rbc_pool.tile([HD, NT], F32, tag="rbc", name=f"rbc{h}_{nt}")
                    src_ = rden_d[pair * 4 + i : pair * 4 + i + 1, :]
                    bsrc = bass.AP(
                        tensor=src_.tensor,
                        offset=src_.offset,
                        ap=[[0, HD], list(src_.ap[-1])],
                    )
                    nc.sync.dma_start(out=rbc, in_=bsrc)
                    ct, prow = h // 2, (h % 2) * HD
                    nc.vector.tensor_mul(
                        aot[ct][prow : prow + HD, nt * NT : (nt + 1) * NT],
                        u_sb[0:HD, :],
                        rbc,
                    )

            prev = None
            for u in units:
                exps = emit_s_exp(u)
                if prev is not None:
                    emit_pv(*prev)
                prev = (u, exps)
            emit_pv(*prev)

        if "pproj" not in _SKIP:
            # ---------- phase 3: output projection + bias ----------
            for dt in range(NNT):
                wp_tiles = []
                for ck in range(NCH):
                    wp_t = wst_pool.tile([P, NT], F32R, tag="wst", name=f"wp{dt}_{ck}")
                    nc.sync.dma_start(
                        wp_t, wpT[ck * P : (ck + 1) * P, dt * NT : (dt + 1) * NT]
                    )
                    wp_tiles.append(wp_t)
                for nt2 in range(NMT):
                    ps_o = psum.tile([P, NT], F32, tag="ps", name=f"pso{dt}_{nt2}")
                    for ck in range(NCH):
                        nc.tensor.matmul(
                            ps_o,
                            lhsT=(aot[ck][:, nt2 * P : (nt2 + 1) * P]),
                            rhs=(wp_tiles[ck]),
                            start=(ck == 0),
                            stop=(ck == NCH - 1),
                        )
                    o_sb = stage_pool.tile([P, NT], F32, tag="stage", name=f"o{dt}_{nt2}")
                    nc.vector.tensor_add(o_sb, ps_o, bias_sb[:, dt * NT : (dt + 1) * NT])
                    nc.sync.dma_start(
                        out[nt2 * P : (nt2 + 1) * P, dt * NT : (dt + 1) * NT], o_sb
                    )

    nc.compile()
    return nc


def make_in_maps(x, w_qkv, w_proj, b_proj):
    wqkT = np.ascontiguousarray(w_qkv[: 2 * C].T)
    wvT = np.ascontiguousarray(w_qkv[2 * C :].T)
    wpT = np.ascontiguousarray(w_proj.T)
    bias_bc = np.ascontiguousarray(np.broadcast_to(b_proj, (P, C)))
    ones = np.ones((P, H), dtype=np.float32)
    in_maps = []
    for b in range(B):
        in_maps.append(
            {
                "xT": np.ascontiguousarray(x[b].T),
                "wqkT": wqkT,
                "wvT": wvT,
                "wpT": wpT,
                "bias_bc": bias_bc,
                "ones_col": ones,
            }
        )
    return in_maps


_CACHED_NC = None


def kernel(x, w_qkv, w_proj, b_proj):
    global _CACHED_NC
    x = np.asarray(x, dtype=np.float32)
    w_qkv = np.asarray(w_qkv, dtype=np.float32)
    w_proj = np.asarray(w_proj, dtype=np.float32)
    b_proj = np.asarray(b_proj, dtype=np.float32)
    if _CACHED_NC is None:
        _CACHED_NC = build_module()
    nc = _CACHED_NC
    in_maps = make_in_maps(x, w_qkv, w_proj, b_proj)
    res = bass_utils.run_bass_kernel_spmd(nc, in_maps, core_ids=list(range(B)))
    return np.stack([res.results[b]["out"] for b in range(B)], axis=0)


if __name__ == "__main__":
    nc = build_module()
    ninst = sum(len(b.instructions) for b in nc.m.functions[0].blocks)
    print("module built ok;", ninst, "instructions")



# revision 27
# speedup vs baseline: 1.2641x; 1.2641x over previous
"""Multi-head attention (B=8, N=1024, C=1024, H=16) on 8 Trainium2 NeuronCores.

Sharding: pure data-parallel — one batch element per core, weights replicated,
no collectives.

v2: all matmuls in bf16 (fp32 matmuls are power-throttled to ~half rate on
TRN2; bf16 runs the PE at full 2.4 GHz). Everything stays SBUF-resident (no
DRAM bounce for qkT). Fused per-head-pair schedule so the ACT engine (exp)
overlaps the projection matmuls. Accumulation is fp32 in PSUM throughout.

Per-core algorithm:
  qk proj   qkT[d, n] = wqkT[c, d].T @ xT[c, n] per head pair, cast bf16.
  v proj    v[m, d] natural layout, interleaved per m-tile as [m, 16*(64+1)]
            with a ones column per head (PV then emits softmax denominators
            for free in PSUM row 64).
  attention per unit (head, nt-half):
            S^T[m, n] = k.T @ q      (keys on partitions, K=64 row-packed)
            E = exp(SCALE * S^T)     (ACT, PSUM -> SBUF bf16)
            U[65, n] = v_aug.T @ E   (row 64 = denominator)
            rden = reciprocal(U[64]) (DVE, fp32)  -> cast bf16
            bc[128, n] = ones_bd.T @ rden   (PE broadcast across partitions)
            aot[c, n] = U[0:64] * bc        (Pool engine, writes bf16)
  proj      out[n, d] = aot[c, n].T @ wpT[c, d] + bias (Pool add), DMA out.
"""

import sys

if "/opt/trn_rl_repo" not in sys.path:
    sys.path.insert(0, "/opt/trn_rl_repo")

from contextlib import ExitStack

import numpy as np

import concourse.bass as bass
import concourse.mybir as mybir
from concourse import bacc
import concourse.tile as tile
from concourse import bass_utils

B, N, C, H = 8, 1024, 1024, 16
HD = C // H          # 64
NP = H // 2          # 8 head pairs
SCALE = HD ** -0.5   # 0.125
P = 128              # SBUF partitions
NT = 512             # psum-bank moving tile
NCH = C // P         # 8 contraction chunks over channels
NMT = N // P         # 8 token tiles of 128
NNT = N // NT        # 2 token tiles of 512
F32 = mybir.dt.float32
BF16 = mybir.dt.bfloat16
EXP = mybir.ActivationFunctionType.Exp


def build_module():
    import os
    _SKIP = set(filter(None, os.environ.get("K_SKIP", "").split(",")))
    nc = bacc.Bacc("TRN2", target_bir_lowering=False, debug=False, num_devices=B)

    xT = nc.dram_tensor("xT", [C, N], BF16, kind="ExternalInput").ap()
    wqkT = nc.dram_tensor("wqkT", [C, 2 * C], BF16, kind="ExternalInput").ap()
    wvT = nc.dram_tensor("wvT", [C, C], BF16, kind="ExternalInput").ap()
    wpT = nc.dram_tensor("wpT", [C, C], BF16, kind="ExternalInput").ap()
    bias = nc.dram_tensor("bias_bc", [P, C], F32, kind="ExternalInput").ap()
    ones_col = nc.dram_tensor("ones_col", [P, H], BF16, kind="ExternalInput").ap()
    ones_bd = nc.dram_tensor("ones_bd", [P, P], BF16, kind="ExternalInput").ap()
    out = nc.dram_tensor("out", [N, C], F32, kind="ExternalOutput").ap()
    _DBG = os.environ.get("K_DEBUG", "")
    dbg = {}
    if _DBG:
        dbg["aot"] = nc.dram_tensor("dbg_aot", [C, N], BF16, kind="ExternalOutput").ap()
        dbg["qk"] = nc.dram_tensor("dbg_qk", [2 * P, N], BF16, kind="ExternalOutput").ap()
        dbg["v"] = nc.dram_tensor("dbg_v", [P, H * (HD + 1)], BF16, kind="ExternalOutput").ap()
        dbg["e"] = nc.dram_tensor("dbg_e", [P, N * NMT // 2], BF16, kind="ExternalOutput").ap()
        dbg["rb"] = nc.dram_tensor("dbg_rb", [P, NT], BF16, kind="ExternalOutput").ap()
        dbg["rc"] = nc.dram_tensor("dbg_rc", [P, NT], F32, kind="ExternalOutput").ap()

    with tile.TileContext(nc) as tc, ExitStack() as ctx:
        xt_pool = ctx.enter_context(tc.tile_pool(name="xt", bufs=NCH))
        wqk_pool = ctx.enter_context(tc.tile_pool(name="wqk", bufs=NCH))
        wv_pool = ctx.enter_context(tc.tile_pool(name="wv", bufs=NCH))
        wp_pool = ctx.enter_context(tc.tile_pool(name="wp", bufs=NCH))
        qk_pool = ctx.enter_context(tc.tile_pool(name="qk", bufs=6))
        vsb_pool = ctx.enter_context(tc.tile_pool(name="vsb", bufs=1))
        e_pool = ctx.enter_context(tc.tile_pool(name="e", bufs=5))
        aot_pool = ctx.enter_context(tc.tile_pool(name="aot", bufs=1))
        one_pool = ctx.enter_context(tc.tile_pool(name="one", bufs=1))
        rden_pool = ctx.enter_context(tc.tile_pool(name="rden", bufs=2))
        osb_pool = ctx.enter_context(tc.tile_pool(name="osb", bufs=2))
        ps_pool = ctx.enter_context(tc.tile_pool(name="ps", bufs=2, space="PSUM"))
        pu_pool = ctx.enter_context(tc.tile_pool(name="pu", bufs=3, space="PSUM"))
        bc_pool = ctx.enter_context(tc.tile_pool(name="bc", bufs=1, space="PSUM"))

        # ---------- input loads ----------
        xts, wvs = [], []
        for t in range(NCH):
            xt_t = xt_pool.tile([P, N], BF16, tag="xt", name=f"xt{t}")
            nc.sync.dma_start(xt_t, xT[t * P : (t + 1) * P, :])
            xts.append(xt_t)
            wv_t = wv_pool.tile([P, C], BF16, tag="wv", name=f"wv{t}")
            nc.sync.dma_start(wv_t, wvT[t * P : (t + 1) * P, :])
            wvs.append(wv_t)
        wqks = []
        for t in range(NCH):
            wqk_t = wqk_pool.tile([P, 2 * C], BF16, tag="wqk", name=f"wqk{t}")
            nc.sync.dma_start(wqk_t, wqkT[t * P : (t + 1) * P, :])
            wqks.append(wqk_t)
        bias_sb = one_pool.tile([P, C], F32, tag="bias", name="bias_sb")
        nc.sync.dma_start(bias_sb, bias)
        onesbd_sb = one_pool.tile([P, P], BF16, tag="obd", name="onesbd_sb")
        nc.sync.dma_start(onesbd_sb, ones_bd)

        # v tiles (natural layout + ones cols), attention-out accumulators
        vsb = []
        for mt in range(NMT):
            v_t = vsb_pool.tile([P, H * (HD + 1)], BF16, tag=f"v{mt}", name=f"v{mt}")
            nc.sync.dma_start(
                v_t.rearrange("p (h w) -> p h w", w=HD + 1)[:, :, HD : HD + 1], ones_col
            )
            vsb.append(v_t)
        aot = []
        for t in range(NCH):
            a_t = aot_pool.tile([P, N], BF16, tag=f"aot{t}", name=f"aot{t}")
            aot.append(a_t)

        # ---------- phase emitters ----------
        def emit_v_mt(mt):
            """v projection for one m-tile: psum [128, 1024] = both d halves."""
            ps_v = ps_pool.tile([P, N], F32, tag="ps", name=f"psv{mt}")
            for ck in range(NCH):
                for hv in range(2):
                    nc.tensor.matmul(
                        ps_v[:, hv * NT : (hv + 1) * NT],
                        lhsT=xts[ck][:, mt * P : (mt + 1) * P],
                        rhs=wvs[ck][:, hv * NT : (hv + 1) * NT],
                        start=(ck == 0),
                        stop=(ck == NCH - 1),
                    )
            dst = vsb[mt].rearrange("p (h w) -> p h w", w=HD + 1)[:, :, 0:HD]
            nc.vector.tensor_copy(dst, ps_v.rearrange("p (h w) -> p h w", w=HD))

        def emit_qk(p):
            """qk projection for head pair p -> bf16 tiles [128, 1024] q and k."""
            res = []
            for which in range(2):  # 0 = q rows, 1 = k rows
                dlo = which * C + p * P
                ps_qk = ps_pool.tile([P, N], F32, tag="ps", name=f"psqk{p}_{which}")
                for nt in range(NNT):
                    for ck in range(NCH):
                        nc.tensor.matmul(
                            ps_qk[:, nt * NT : (nt + 1) * NT],
                            lhsT=wqks[ck][:, dlo : dlo + P],
                            rhs=xts[ck][:, nt * NT : (nt + 1) * NT],
                            start=(ck == 0),
                            stop=(ck == NCH - 1),
                        )
                sb = qk_pool.tile([P, N], BF16, tag="qk", name=f"qk{p}_{which}")
                nc.vector.tensor_copy(sb, ps_qk)
                res.append(sb)
            return res  # [q_tile, k_tile]

        def emit_s_exp(u):
            """S^T matmuls + exp for one unit (pair p, head-slot j, nt)."""
            (p, j, nt, qt, kt) = u
            pl = slice(j * HD, (j + 1) * HD)
            e_t = e_pool.tile([P, N * NMT // 2], BF16, tag="e", name=f"e{p}_{j}_{nt}")
            for g in range(4):  # two m-chunks per psum tile
                ps_s = ps_pool.tile([P, N], F32, tag="ps", name=f"pss{p}_{j}_{nt}_{g}")
                for half in range(2):
                    mc = 2 * g + half
                    nc.tensor.matmul(
                        ps_s[:, half * NT : (half + 1) * NT],
                        lhsT=kt[pl, mc * P : (mc + 1) * P],
                        rhs=qt[pl, nt * NT : (nt + 1) * NT],
                        start=True,
                        stop=True,
                    )
                nc.scalar.activation(
                    e_t[:, g * N : (g + 1) * N], ps_s, EXP, scale=SCALE
                )
            return e_t

        pair_nt_state = {}

        def emit_pv(u, e_t):
            """PV + denominator reciprocal; on nt-group completion, broadcast
            + normalize into aot."""
            (p, j, nt, qt, kt) = u
            h = 2 * p + j
            ps_u = pu_pool.tile([HD + 1, NT], F32, tag="pu", name=f"psu{h}_{nt}")
            for mc in range(NMT):
                nc.tensor.matmul(
                    ps_u,
                    lhsT=vsb[mc][:, h * (HD + 1) : (h + 1) * (HD + 1)],
                    rhs=e_t[:, mc * NT : (mc + 1) * NT],
                    start=(mc == 0),
                    stop=(mc == NMT - 1),
                )
            key = (p, nt)
            if key not in pair_nt_state:
                rb = rden_pool.tile([P, NT], BF16, tag="rb", name=f"rb{p}_{nt}")
                nc.gpsimd.memset(rb, 0.0)
                rf = rden_pool.tile([P, NT], F32, tag="rf", name=f"rf{p}_{nt}")
                nc.gpsimd.memset(rf, 1.0)
                pair_nt_state[key] = {
                    "rf": rf,
                    "rr": rden_pool.tile([P, NT], F32, tag="rr", name=f"rr{p}_{nt}"),
                    "rb": rb,
                    "us": [],
                }
            st = pair_nt_state[key]
            # stage this unit's denominator row (PSUM row 64 -> rf row 64*j)
            nc.scalar.copy(st["rf"][j * HD : j * HD + 1, :], ps_u[HD : HD + 1, :])
            st["us"].append((j, ps_u))
            if len(st["us"]) == 2:
                pair_nt_state.pop(key)
                nc.vector.reciprocal(out=st["rr"], in_=st["rf"])
                for (jj, _) in st["us"]:
                    nc.vector.tensor_copy(
                        st["rb"][jj * HD : jj * HD + 1, :],
                        st["rr"][jj * HD : jj * HD + 1, :],
                    )
                bc = bc_pool.tile([P, NT], F32, tag="bc", name=f"bc{p}_{nt}")
                nc.tensor.matmul(
                    bc,
                    lhsT=onesbd_sb[0 : HD + 1, :],
                    rhs=st["rb"][0 : HD + 1, :],
                    start=True,
                    stop=True,
                )
                rbc = rden_pool.tile([HD, 2 * NT], F32, tag="rc", name=f"rc{p}_{nt}")
                nc.scalar.copy(rbc[:, 0:NT], bc[0:HD, :])
                nc.scalar.copy(rbc[:, NT : 2 * NT], bc[HD : 2 * HD, :])
                if _DBG and p == 0 and nt == 0:
                    nc.sync.dma_start(dbg["rb"], st["rb"])
                    nc.sync.dma_start(dbg["rc"][0:HD, :], rbc[:, 0:NT])
                for (jj, psu) in st["us"]:
                    nc.vector.tensor_mul(
                        aot[p][jj * HD : (jj + 1) * HD, nt * NT : (nt + 1) * NT],
                        psu[0:HD, :],
                        rbc[:, jj * NT : (jj + 1) * NT],
                    )

        # ---------- fused schedule ----------
        wps = []
        if "pattn" in _SKIP:
            for t in range(NCH):
                nc.sync.dma_start(aot[t], xT[t * P : (t + 1) * P, :])
                wp_t = wp_pool.tile([P, C], BF16, tag="wp", name=f"wp{t}")
                nc.sync.dma_start(wp_t, wpT[t * P : (t + 1) * P, :])
                wps.append(wp_t)
        else:
            qk_tiles = emit_qk(0)
            if _DBG:
                nc.sync.dma_start(dbg["qk"][0:P, :], qk_tiles[0])
                nc.sync.dma_start(dbg["qk"][P : 2 * P, :], qk_tiles[1])
            units = []  # queue of (unit, e_t) awaiting PV
            for p in range(NP):
                nqk = None
                for s, (nt, j) in enumerate([(0, 0), (0, 1), (1, 0), (1, 1)]):
                    u = (p, j, nt, qk_tiles[0], qk_tiles[1])
                    e_t = emit_s_exp(u)
                    if _DBG and p == 0 and j == 0 and nt == 0:
                        nc.sync.dma_start(dbg["e"], e_t)
                    if p == 0:
                        # v projection rides between the first pair's S units;
                        # PV must wait until every v tile exists.
                        emit_v_mt(2 * s)
                        emit_v_mt(2 * s + 1)
                    else:
                        emit_pv(*units.pop(0))
                        if len(units) > 4:
                            emit_pv(*units.pop(0))
                    units.append((u, e_t))
                if p + 1 < NP:
                    nqk = emit_qk(p + 1)
                if p == NP - 2:
                    for t in range(NCH):
                        wp_t = wp_pool.tile([P, C], BF16, tag="wp", name=f"wp{t}")
                        nc.sync.dma_start(wp_t, wpT[t * P : (t + 1) * P, :])
                        wps.append(wp_t)
                if nqk is not None:
                    qk_tiles = nqk
            while units:
                emit_pv(*units.pop(0))
        if _DBG:
            for t in range(NCH):
                nc.sync.dma_start(dbg["aot"][t * P : (t + 1) * P, :], aot[t])
            nc.sync.dma_start(dbg["v"], vsb[0])

        # ---------- output projection + bias ----------
        if "pproj" not in _SKIP:
            for nt2 in range(NMT):
                ps_o = ps_pool.tile([P, C], F32, tag="ps", name=f"pso{nt2}")
                for ck in range(NCH):
                    for dt in range(2):
                        nc.tensor.matmul(
                            ps_o[:, dt * NT : (dt + 1) * NT],
                            lhsT=aot[ck][:, nt2 * P : (nt2 + 1) * P],
                            rhs=wps[ck][:, dt * NT : (dt + 1) * NT],
                            start=(ck == 0),
                            stop=(ck == NCH - 1),
                        )
                o_sb = osb_pool.tile([P, C], F32, tag="osb", name=f"o{nt2}")
                nc.vector.tensor_add(o_sb, ps_o, bias_sb)
                nc.sync.dma_start(out[nt2 * P : (nt2 + 1) * P, :], o_sb)

    nc.compile()
    return nc


def make_in_maps(x, w_qkv, w_proj, b_proj):
    import ml_dtypes

    bf = ml_dtypes.bfloat16
    wqkT = np.ascontiguousarray(w_qkv[: 2 * C].T.astype(bf))
    wvT = np.ascontiguousarray(w_qkv[2 * C :].T.astype(bf))
    wpT = np.ascontiguousarray(w_proj.T.astype(bf))
    bias_bc = np.ascontiguousarray(np.broadcast_to(b_proj, (P, C)).astype(np.float32))
    ones = np.ones((P, H), dtype=bf)
    obd = np.zeros((P, P), dtype=bf)
    obd[0, :HD] = 1
    obd[HD, HD:] = 1
    in_maps = []
    for b in range(B):
        in_maps.append(
            {
                "xT": np.ascontiguousarray(x[b].T.astype(bf)),
                "wqkT": wqkT,
                "wvT": wvT,
                "wpT": wpT,
                "bias_bc": bias_bc,
                "ones_col": ones,
                "ones_bd": obd,
            }
        )
    return in_maps


_CACHED_NC = None


def kernel(x, w_qkv, w_proj, b_proj):
    global _CACHED_NC
    x = np.asarray(x, dtype=np.float32)
    w_qkv = np.asarray(w_qkv, dtype=np.float32)
    w_proj = np.asarray(w_proj, dtype=np.float32)
    b_proj = np.asarray(b_proj, dtype=np.float32)
    if _CACHED_NC is None:
        _CACHED_NC = build_module()
    nc = _CACHED_NC
    in_maps = make_in_maps(x, w_qkv, w_proj, b_proj)
    res = bass_utils.run_bass_kernel_spmd(nc, in_maps, core_ids=list(range(B)))
    return np.stack([res.results[b]["out"] for b in range(B)], axis=0)


if __name__ == "__main__":
    nc = build_module()
    ninst = sum(len(b.instructions) for b in nc.m.functions[0].blocks)
    print("module built ok;", ninst, "instructions")


# revision 29
# speedup vs baseline: 1.6191x; 1.2809x over previous
"""Multi-head attention (B=8, N=1024, C=1024, H=16) on 8 Trainium2 NeuronCores.

Sharding: pure data-parallel — one batch element per core, weights replicated,
no collectives.

v2: all matmuls in bf16 (fp32 matmuls are power-throttled to ~half rate on
TRN2; bf16 runs the PE at full 2.4 GHz). Everything stays SBUF-resident (no
DRAM bounce for qkT). Fused per-head-pair schedule so the ACT engine (exp)
overlaps the projection matmuls. Accumulation is fp32 in PSUM throughout.

Per-core algorithm:
  qk proj   qkT[d, n] = wqkT[c, d].T @ xT[c, n] per head pair, cast bf16.
  v proj    v[m, d] natural layout, interleaved per m-tile as [m, 16*(64+1)]
            with a ones column per head (PV then emits softmax denominators
            for free in PSUM row 64).
  attention per unit (head, nt-half):
            S^T[m, n] = k.T @ q      (keys on partitions, K=64 row-packed)
            E = exp(SCALE * S^T)     (ACT, PSUM -> SBUF bf16)
            U[65, n] = v_aug.T @ E   (row 64 = denominator)
            rden = reciprocal(U[64]) (DVE, fp32)  -> cast bf16
            bc[128, n] = ones_bd.T @ rden   (PE broadcast across partitions)
            aot[c, n] = U[0:64] * bc        (Pool engine, writes bf16)
  proj      out[n, d] = aot[c, n].T @ wpT[c, d] + bias (Pool add), DMA out.
"""

import sys

if "/opt/trn_rl_repo" not in sys.path:
    sys.path.insert(0, "/opt/trn_rl_repo")

from contextlib import ExitStack

import numpy as np

import concourse.bass as bass
import concourse.mybir as mybir
from concourse import bacc
import concourse.tile as tile
from concourse import bass_utils

B, N, C, H = 8, 1024, 1024, 16
HD = C // H          # 64
NP = H // 2          # 8 head pairs
SCALE = HD ** -0.5   # 0.125
P = 128              # SBUF partitions
NT = 512             # psum-bank moving tile
NCH = C // P         # 8 contraction chunks over channels
NMT = N // P         # 8 token tiles of 128
NNT = N // NT        # 2 token tiles of 512
F32 = mybir.dt.float32
BF16 = mybir.dt.bfloat16
EXP = mybir.ActivationFunctionType.Exp


def build_module():
    import os
    _SKIP = set(filter(None, os.environ.get("K_SKIP", "").split(",")))
    nc = bacc.Bacc("TRN2", target_bir_lowering=False, debug=False, num_devices=B)

    xT = nc.dram_tensor("xT", [C, N], BF16, kind="ExternalInput").ap()
    wqkT = nc.dram_tensor("wqkT", [C, 2 * C], BF16, kind="ExternalInput").ap()
    wvT = nc.dram_tensor("wvT", [C, C], BF16, kind="ExternalInput").ap()
    wpT = nc.dram_tensor("wpT", [C, C], BF16, kind="ExternalInput").ap()
    bias = nc.dram_tensor("bias_bc", [P, C], F32, kind="ExternalInput").ap()
    ones_col = nc.dram_tensor("ones_col", [P, H], BF16, kind="ExternalInput").ap()
    ones_bd = nc.dram_tensor("ones_bd", [P, P], BF16, kind="ExternalInput").ap()
    out = nc.dram_tensor("out", [N, C], F32, kind="ExternalOutput").ap()
    _DBG = os.environ.get("K_DEBUG", "")
    dbg = {}
    if _DBG:
        dbg["aot"] = nc.dram_tensor("dbg_aot", [C, N], BF16, kind="ExternalOutput").ap()
        dbg["qk"] = nc.dram_tensor("dbg_qk", [2 * P, N], BF16, kind="ExternalOutput").ap()
        dbg["v"] = nc.dram_tensor("dbg_v", [P, H * (HD + 1)], BF16, kind="ExternalOutput").ap()
        dbg["e"] = nc.dram_tensor("dbg_e", [P, N * NMT // 2], BF16, kind="ExternalOutput").ap()
        dbg["rb"] = nc.dram_tensor("dbg_rb", [P, NT], BF16, kind="ExternalOutput").ap()
        dbg["rc"] = nc.dram_tensor("dbg_rc", [P, NT], F32, kind="ExternalOutput").ap()

    with tile.TileContext(nc) as tc, ExitStack() as ctx:
        xt_pool = ctx.enter_context(tc.tile_pool(name="xt", bufs=NCH))
        wqk_pool = ctx.enter_context(tc.tile_pool(name="wqk", bufs=NCH))
        wv_pool = ctx.enter_context(tc.tile_pool(name="wv", bufs=NCH))
        wp_pool = ctx.enter_context(tc.tile_pool(name="wp", bufs=NCH))
        qk_pool = ctx.enter_context(tc.tile_pool(name="qk", bufs=6))
        vsb_pool = ctx.enter_context(tc.tile_pool(name="vsb", bufs=1))
        e_pool = ctx.enter_context(tc.tile_pool(name="e", bufs=5))
        aot_pool = ctx.enter_context(tc.tile_pool(name="aot", bufs=1))
        one_pool = ctx.enter_context(tc.tile_pool(name="one", bufs=1))
        rden_pool = ctx.enter_context(tc.tile_pool(name="rden", bufs=2))
        osb_pool = ctx.enter_context(tc.tile_pool(name="osb", bufs=2))
        ps_pool = ctx.enter_context(tc.tile_pool(name="ps", bufs=2, space="PSUM"))
        pu_pool = ctx.enter_context(tc.tile_pool(name="pu", bufs=3, space="PSUM"))
        bc_pool = ctx.enter_context(tc.tile_pool(name="bc", bufs=1, space="PSUM"))

        # ---------- input loads ----------
        xts, wvs = [], []
        for t in range(NCH):
            xt_t = xt_pool.tile([P, N], BF16, tag="xt", name=f"xt{t}")
            nc.sync.dma_start(xt_t, xT[t * P : (t + 1) * P, :])
            xts.append(xt_t)
            wv_t = wv_pool.tile([P, C], BF16, tag="wv", name=f"wv{t}")
            nc.sync.dma_start(wv_t, wvT[t * P : (t + 1) * P, :])
            wvs.append(wv_t)
        wqks = []
        for t in range(NCH):
            wqk_t = wqk_pool.tile([P, 2 * C], BF16, tag="wqk", name=f"wqk{t}")
            nc.sync.dma_start(wqk_t, wqkT[t * P : (t + 1) * P, :])
            wqks.append(wqk_t)
        bias_sb = one_pool.tile([P, C], F32, tag="bias", name="bias_sb")
        nc.sync.dma_start(bias_sb, bias)
        onesbd_sb = one_pool.tile([P, P], BF16, tag="obd", name="onesbd_sb")
        nc.sync.dma_start(onesbd_sb, ones_bd)

        # v tiles (natural layout + ones cols), attention-out accumulators
        vsb = []
        for mt in range(NMT):
            v_t = vsb_pool.tile([P, H * (HD + 1)], BF16, tag=f"v{mt}", name=f"v{mt}")
            nc.sync.dma_start(
                v_t.rearrange("p (h w) -> p h w", w=HD + 1)[:, :, HD : HD + 1], ones_col
            )
            vsb.append(v_t)
        aot = []
        for t in range(NCH):
            a_t = aot_pool.tile([P, N], BF16, tag=f"aot{t}", name=f"aot{t}")
            aot.append(a_t)

        # ---------- phase emitters ----------
        def emit_v_mt(mt):
            """v projection for one m-tile: psum [128, 1024] = both d halves."""
            ps_v = ps_pool.tile([P, N], F32, tag="ps", name=f"psv{mt}")
            for ck in range(NCH):
                for hv in range(2):
                    nc.tensor.matmul(
                        ps_v[:, hv * NT : (hv + 1) * NT],
                        lhsT=xts[ck][:, mt * P : (mt + 1) * P],
                        rhs=wvs[ck][:, hv * NT : (hv + 1) * NT],
                        start=(ck == 0),
                        stop=(ck == NCH - 1),
                    )
            dst = vsb[mt].rearrange("p (h w) -> p h w", w=HD + 1)[:, :, 0:HD]
            nc.vector.tensor_copy(dst, ps_v.rearrange("p (h w) -> p h w", w=HD))

        def emit_qk(p):
            """qk projection for head pair p -> bf16 tiles [128, 1024] q and k."""
            res = []
            for which in range(2):  # 0 = q rows, 1 = k rows
                dlo = which * C + p * P
                ps_qk = ps_pool.tile([P, N], F32, tag="ps", name=f"psqk{p}_{which}")
                for nt in range(NNT):
                    for ck in range(NCH):
                        nc.tensor.matmul(
                            ps_qk[:, nt * NT : (nt + 1) * NT],
                            lhsT=wqks[ck][:, dlo : dlo + P],
                            rhs=xts[ck][:, nt * NT : (nt + 1) * NT],
                            start=(ck == 0),
                            stop=(ck == NCH - 1),
                        )
                sb = qk_pool.tile([P, N], BF16, tag="qk", name=f"qk{p}_{which}")
                nc.vector.tensor_copy(sb, ps_qk)
                res.append(sb)
            return res  # [q_tile, k_tile]

        def emit_s_exp(u):
            """S^T matmuls + exp for one unit (pair p, head-slot j, nt)."""
            (p, j, nt, qt, kt) = u
            pl = slice(j * HD, (j + 1) * HD)
            e_t = e_pool.tile([P, N * NMT // 2], BF16, tag="e", name=f"e{p}_{j}_{nt}")
            for g in range(4):  # two m-chunks per psum tile
                ps_s = ps_pool.tile([P, N], F32, tag="ps", name=f"pss{p}_{j}_{nt}_{g}")
                for half in range(2):
                    mc = 2 * g + half
                    nc.tensor.matmul(
                        ps_s[:, half * NT : (half + 1) * NT],
                        lhsT=kt[pl, mc * P : (mc + 1) * P],
                        rhs=qt[pl, nt * NT : (nt + 1) * NT],
                        start=True,
                        stop=True,
                    )
                nc.scalar.activation(
                    e_t[:, g * N : (g + 1) * N], ps_s, EXP, scale=SCALE
                )
            return e_t

        pair_nt_state = {}

        def emit_pv(u, e_t):
            """PV + denominator reciprocal; on nt-group completion, broadcast
            + normalize into aot."""
            (p, j, nt, qt, kt) = u
            h = 2 * p + j
            ps_u = pu_pool.tile([HD + 1, NT], F32, tag="pu", name=f"psu{h}_{nt}")
            for mc in range(NMT):
                nc.tensor.matmul(
                    ps_u,
                    lhsT=vsb[mc][:, h * (HD + 1) : (h + 1) * (HD + 1)],
                    rhs=e_t[:, mc * NT : (mc + 1) * NT],
                    start=(mc == 0),
                    stop=(mc == NMT - 1),
                )
            key = (p, nt)
            if key not in pair_nt_state:
                rb = rden_pool.tile([P, NT], BF16, tag="rb", name=f"rb{p}_{nt}")
                nc.gpsimd.memset(rb, 0.0)
                pair_nt_state[key] = {"rb": rb, "us": []}
            st = pair_nt_state[key]
            # ACT-engine reciprocal of the denominator row, straight from PSUM
            # into the bf16 broadcast-source tile (rel err ~1e-5, fine here).
            se = nc.scalar
            ins = [se.lower_ap(ps_u[HD : HD + 1, :])]
            for val in (0.0, 1.0, 0.0):
                ins.append(mybir.ImmediateValue(dtype=mybir.dt.float32, value=val))
            se.add_instruction(
                mybir.InstActivation(
                    name=nc.get_next_instruction_name(),
                    func=mybir.ActivationFunctionType.Reciprocal,
                    ins=ins,
                    outs=[se.lower_ap(st["rb"][j * HD : j * HD + 1, :])],
                )
            )
            st["us"].append((j, ps_u))
            if len(st["us"]) == 2:
                pair_nt_state.pop(key)
                bc = bc_pool.tile([P, NT], F32, tag="bc", name=f"bc{p}_{nt}")
                nc.tensor.matmul(
                    bc,
                    lhsT=onesbd_sb[0 : HD + 1, :],
                    rhs=st["rb"][0 : HD + 1, :],
                    start=True,
                    stop=True,
                )
                rbc = rden_pool.tile([HD, 2 * NT], F32, tag="rc", name=f"rc{p}_{nt}")
                nc.vector.tensor_copy(rbc[:, 0:NT], bc[0:HD, :])
                nc.vector.tensor_copy(rbc[:, NT : 2 * NT], bc[HD : 2 * HD, :])
                if _DBG and p == 0 and nt == 0:
                    nc.sync.dma_start(dbg["rb"], st["rb"])
                    nc.sync.dma_start(dbg["rc"][0:HD, :], rbc[:, 0:NT])
                for (jj, psu) in st["us"]:
                    nc.vector.tensor_mul(
                        aot[p][jj * HD : (jj + 1) * HD, nt * NT : (nt + 1) * NT],
                        psu[0:HD, :],
                        rbc[:, jj * NT : (jj + 1) * NT],
                    )

        # ---------- fused schedule ----------
        wps = []
        if "pattn" in _SKIP:
            for t in range(NCH):
                nc.sync.dma_start(aot[t], xT[t * P : (t + 1) * P, :])
                wp_t = wp_pool.tile([P, C], BF16, tag="wp", name=f"wp{t}")
                nc.sync.dma_start(wp_t, wpT[t * P : (t + 1) * P, :])
                wps.append(wp_t)
        else:
            qk_tiles = emit_qk(0)
            if _DBG:
                nc.sync.dma_start(dbg["qk"][0:P, :], qk_tiles[0])
                nc.sync.dma_start(dbg["qk"][P : 2 * P, :], qk_tiles[1])
            units = []  # queue of (unit, e_t) awaiting PV
            for p in range(NP):
                nqk = None
                for s, (nt, j) in enumerate([(0, 0), (0, 1), (1, 0), (1, 1)]):
                    u = (p, j, nt, qk_tiles[0], qk_tiles[1])
                    e_t = emit_s_exp(u)
                    if _DBG and p == 0 and j == 0 and nt == 0:
                        nc.sync.dma_start(dbg["e"], e_t)
                    if p == 0:
                        # v projection rides between the first pair's S units;
                        # PV must wait until every v tile exists.
                        emit_v_mt(2 * s)
                        emit_v_mt(2 * s + 1)
                    else:
                        emit_pv(*units.pop(0))
                        if len(units) > 4:
                            emit_pv(*units.pop(0))
                    units.append((u, e_t))
                if p + 1 < NP:
                    nqk = emit_qk(p + 1)
                if p == NP - 2:
                    for t in range(NCH):
                        wp_t = wp_pool.tile([P, C], BF16, tag="wp", name=f"wp{t}")
                        nc.sync.dma_start(wp_t, wpT[t * P : (t + 1) * P, :])
                        wps.append(wp_t)
                if nqk is not None:
                    qk_tiles = nqk
            while units:
                emit_pv(*units.pop(0))
        if _DBG:
            for t in range(NCH):
                nc.sync.dma_start(dbg["aot"][t * P : (t + 1) * P, :], aot[t])
            nc.sync.dma_start(dbg["v"], vsb[0])

        # ---------- output projection + bias ----------
        if "pproj" not in _SKIP:
            for nt2 in range(NMT):
                ps_o = ps_pool.tile([P, C], F32, tag="ps", name=f"pso{nt2}")
                for ck in range(NCH):
                    for dt in range(2):
                        nc.tensor.matmul(
                            ps_o[:, dt * NT : (dt + 1) * NT],
                            lhsT=aot[ck][:, nt2 * P : (nt2 + 1) * P],
                            rhs=wps[ck][:, dt * NT : (dt + 1) * NT],
                            start=(ck == 0),
                            stop=(ck == NCH - 1),
                        )
                o_sb = osb_pool.tile([P, C], F32, tag="osb", name=f"o{nt2}")
                nc.vector.tensor_add(o_sb, ps_o, bias_sb)
                nc.sync.dma_start(out[nt2 * P : (nt2 + 1) * P, :], o_sb)

    nc.compile()
    return nc


def make_in_maps(x, w_qkv, w_proj, b_proj):
    import ml_dtypes

    bf = ml_dtypes.bfloat16
    wqkT = np.ascontiguousarray(w_qkv[: 2 * C].T.astype(bf))
    wvT = np.ascontiguousarray(w_qkv[2 * C :].T.astype(bf))
    wpT = np.ascontiguousarray(w_proj.T.astype(bf))
    bias_bc = np.ascontiguousarray(np.broadcast_to(b_proj, (P, C)).astype(np.float32))
    ones = np.ones((P, H), dtype=bf)
    obd = np.zeros((P, P), dtype=bf)
    obd[0, :HD] = 1
    obd[HD, HD:] = 1
    in_maps = []
    for b in range(B):
        in_maps.append(
            {
                "xT": np.ascontiguousarray(x[b].T.astype(bf)),
                "wqkT": wqkT,
                "wvT": wvT,
                "wpT": wpT,
                "bias_bc": bias_bc,
                "ones_col": ones,
                "ones_bd": obd,
            }
        )
    return in_maps


_CACHED_NC = None


def kernel(x, w_qkv, w_proj, b_proj):
    global _CACHED_NC
    x = np.asarray(x, dtype=np.float32)
    w_qkv = np.asarray(w_qkv, dtype=np.float32)
    w_proj = np.asarray(w_proj, dtype=np.float32)
    b_proj = np.asarray(b_proj, dtype=np.float32)
    if _CACHED_NC is None:
        _CACHED_NC = build_module()
    nc = _CACHED_NC
    in_maps = make_in_maps(x, w_qkv, w_proj, b_proj)
    res = bass_utils.run_bass_kernel_spmd(nc, in_maps, core_ids=list(range(B)))
    return np.stack([res.results[b]["out"] for b in range(B)], axis=0)


if __name__ == "__main__":
    nc = build_module()
    ninst = sum(len(b.instructions) for b in nc.m.functions[0].blocks)
    print("module built ok;", ninst, "instructions")
